# revision 32
# baseline (speedup 1.0000x reference)
"""Trainium2 Bass kernel for nn_Block_41077067219413.

Reference computation (B=2048, D=dim_in=4096, J=dim_out=4096):
    xf = x.astype(f32)                 # (B, D) in {0,1}
    mf = masks.astype(f32)             # (D, J) in {0,1}
    sums = xf @ mf + (1-xf) @ (1-mf)   # XNOR popcount over D
    out  = sums > thresholds[None, :]  # (B, J) bool

Identity: with x' = 2x-1 in {-1,+1}, m in {0,1}, A = x' @ m:
    sums = A + D - rowsum_x   (colsum terms cancel)
    out  = A > th[j] + rowsum_x[b] - D

Sharding: 4 batch groups x 2 j-halves across 8 cores.  Per core one fp8
DoubleRow GEMM [512 x 4096] @ [4096 x 2048] -- 256 matmuls of
[K=256]x[N=512], the PE-array floor (~55us at 157 TF/s fp8-DR).
Everything else is kept off the PE:
  - x is host-marshalled to the exact stationary tile layout (fp8 +-1,
    transposed, DR k-pairing) -- no on-device transposes/converts.
  - masks DMA'd raw as uint8 in k-pair tile layout, bitcast to fp8
    (byte 0x01 == eps = 2^-9 subnormal); psum accumulates eps*A exactly.
  - thresholds ship as an eps-scaled f32 broadcast tile; epilogue is
    tmp = psum - eps*th (DVE, releases the psum bank) then
    out = tmp > eps*(rowsum-D) (scalar/gpsimd), all integer-exact.
  - rowsum_x comes from a row-major fp8 x copy via accum-reductions
    spread over the scalar/vector/gpsimd engines mid-flight.
PSUM: two waves of 8 banks (2 b-tiles x 4 j-tiles each), kp 0..11
kp-major (tiles consumed in DMA arrival order), kp 12..15 group-major
so groups retire staggered.  Dummy warm-up matmuls ramp the PE p-state
while the first tiles land.
"""

import numpy as np

B, D, J = 2048, 4096, 4096
NCORES = 8
GB = 4                    # batch groups
GJ = 2                    # j halves
ML = B // GB              # 512 rows per core
JL = J // GJ              # 2048 cols per core
P = 128
NB = ML // P              # 4 b-tiles per core
KP = D // 256             # 16 k-pair tiles
JN = 512                  # one PSUM bank
JT = JL // JN             # 4 j-tiles
KRET = 4                  # retirement kps (12..15)
WARM = 30                 # PE p-state warm-up matmuls
# mask DMA chunks (in kp units): small first chunks for a fast start,
# 2-kp chunks after to keep the issue count low
MCHUNKS = (1, 1, 2, 2, 2, 2, 2, 2, 2)

_cache = {}


def _build():
    import concourse.bacc as bacc
    import concourse.mybir as mybir
    import concourse.tile as tile

    dt = mybir.dt
    f8 = dt.float8e4
    f32 = dt.float32
    AF = mybir.ActivationFunctionType
    ALU = mybir.AluOpType
    DR = mybir.MatmulPerfMode.DoubleRow

    nc = bacc.Bacc("TRN2", target_bir_lowering=False, debug=False,
                   num_devices=NCORES)

    xT_d = nc.dram_tensor("xT", [NB, P, KP, 2, P], f8, kind="ExternalInput")
    xrm_d = nc.dram_tensor("xrm", [NB, P, D], f8, kind="ExternalInput")
    m_d = nc.dram_tensor("masks", [KP, P, 2, JL], dt.uint8,
                         kind="ExternalInput")
    thb_d = nc.dram_tensor("thb", [P, JL], f32, kind="ExternalInput")
    o_d = nc.dram_tensor("out", [NB, P, JL], dt.uint8, kind="ExternalOutput")

    with tile.TileContext(nc) as tc:
        with (
            tc.tile_pool(name="const", bufs=1) as constp,
            tc.tile_pool(name="mask", bufs=1) as maskp,
            tc.tile_pool(name="xt", bufs=1) as xtp,
            tc.tile_pool(name="xrm", bufs=1) as xrmp,
            tc.tile_pool(name="acts", bufs=1) as actp,
            tc.tile_pool(name="tmp", bufs=1) as tmpp,
            tc.tile_pool(name="bound", bufs=1) as boundp,
            tc.tile_pool(name="ob", bufs=1) as obsp,
        ):
            # ---- input DMAs on two issue queues.  gpsimd carries the
            # masks in kp (= bulk consumption) order with thb slotted in;
            # scalar carries the two wave-0 xT tiles in parallel.  The
            # late traffic (xrm, wave-1 xT) is gated behind the last mask
            # chunk by a tiny dependent copy so its transfers cannot
            # steal mask bandwidth mid-wave.
            mtc = [maskp.tile([P, nk, 2, JL], dt.uint8, name=f"mkc{c}")
                   for c, nk in enumerate(MCHUNKS)]
            kp2c = []
            for c, nk in enumerate(MCHUNKS):
                kp2c += [(c, i) for i in range(nk)]
            xT01 = xtp.tile([P, 2, KP, 2, P], f8)
            xT23 = xtp.tile([P, 2, KP, 2, P], f8)
            xrm01 = xrmp.tile([P, 2, D], f8)
            xrm23 = xrmp.tile([P, 2, D], f8)
            thb = constp.tile([P, JL], f32)

            def mte(kp, jj):
                c, i = kp2c[kp]
                return mtc[c][:, i, :, jj:jj + JN]

            def xte(b, kp):
                return (xT01[:, b, kp] if b < 2
                        else xT23[:, b - 2, kp])

            # first xT chunk on the scalar queue (lands in parallel with
            # the first mask chunk); everything else rides one strictly
            # ordered gpsimd queue in consumption order, with the later
            # xT chunks slotted just before the kp range that needs them.
            def xtdma(q):
                k0, k1 = 4 * q, 4 * (q + 1)
                eng = nc.scalar if q == 0 else nc.gpsimd
                eng.dma_start(
                    xT01[:, :, k0:k1],
                    xT_d[0:2, :, k0:k1].rearrange("b p k a m -> p b k a m"))

            xtdma(0)
            for c, nk in enumerate(MCHUNKS):
                kp0 = kp2c.index((c, 0))
                if kp0 in (4, 8, 12):
                    xtdma(kp0 // 4)
                nc.gpsimd.dma_start(
                    mtc[c][:], m_d[kp0:kp0 + nk].rearrange(
                        "k p a j -> p k a j"))
            # late traffic rides behind the mask stream in FIFO order
            nc.gpsimd.dma_start(thb[:], thb_d[:])
            nc.gpsimd.dma_start(
                xT23[:], xT_d[2:4].rearrange("b p k a m -> p b k a m"))
            nc.gpsimd.dma_start(
                xrm01[:], xrm_d[0:2].rearrange("b p k -> p b k"))
            nc.gpsimd.dma_start(
                xrm23[:], xrm_d[2:4].rearrange("b p k -> p b k"))

            # ---- constants / warm-up
            wtile = constp.tile([P, 2, P], f8)
            nc.vector.memset(wtile[:], 0.0)
            zero1 = constp.tile([P, 1], f32)
            nc.vector.memset(zero1[:], 0.0)
            neg4 = constp.tile([P, 1], f32)
            nc.vector.memset(neg4[:], -4.0)
            actw = constp.tile([P, 1], f32)
            nc.scalar.activation(actw[:], zero1[:], AF.Identity,
                                 bias=zero1[:], scale=1.0)

            rxa = [constp.tile([P, 1], f32, name=f"rxa{b}")
                   for b in range(NB)]
            rxe = [constp.tile([P, 1], f32, name=f"rxe{b}")
                   for b in range(NB)]
            sc8 = [actp.tile([P, D], f8, name=f"sc8_{i}") for i in range(3)]

            # rowsum reductions: b0/b1 serial on scalar, b2/b3 on vector
            # (emitted later, after the wave-0 psum-releasing ops).
            for b in (0, 1):
                nc.scalar.activation(sc8[b][:], xrm01[:, b], AF.Identity,
                                     bias=zero1[:], scale=1.0,
                                     accum_out=rxa[b][:])
                nc.scalar.activation(rxe[b][:], rxa[b][:], AF.Identity,
                                     bias=neg4[:], scale=1.0 / 1024.0)

            obs = [obsp.tile([P, JL], dt.uint8, name=f"ob{b}")
                   for b in range(NB)]

            with tc.tile_pool(name="psacc", bufs=1, space="PSUM") as psacc:
                dps = psacc.tile([P, JN], f32, tag="acc0", name="dps")
                for i in range(WARM):
                    nc.tensor.matmul(dps[:, 0:P], wtile[:], wtile[:],
                                     start=True, stop=True, perf_mode=DR)

                for w in range(2):
                    bs = (2 * w, 2 * w + 1)
                    ps = {}
                    for b2 in range(2):
                        for j4 in range(JT):
                            ps[(b2, j4)] = psacc.tile(
                                [P, JN], f32, tag=f"acc{b2 * JT + j4}",
                                name=f"acc_w{w}_{b2}_{j4}")
                    # bulk: kp-major over kp 0..11
                    for kp in range(KP - KRET):
                        for b2 in range(2):
                            wap = xte(bs[b2], kp)
                            for j4 in range(JT):
                                nc.tensor.matmul(
                                    ps[(b2, j4)][:], wap,
                                    mte(kp, j4 * JN).bitcast(f8),
                                    start=(kp == 0), stop=False,
                                    perf_mode=DR)
                    # retirement: group-major over kp 12..15, staggered
                    tmps = []
                    for b2 in range(2):
                        b = bs[b2]
                        for j4 in range(JT):
                            jj = j4 * JN
                            for kp in range(KP - KRET, KP):
                                nc.tensor.matmul(
                                    ps[(b2, j4)][:], xte(b, kp),
                                    mte(kp, jj).bitcast(f8),
                                    start=False, stop=(kp == KP - 1),
                                    perf_mode=DR)
                            if w == 0:
                                # two-op epilogue: op1 (DVE) releases the
                                # psum bank using only thb; op2 (deferred
                                # below) waits on the rowsum path.
                                tmp = tmpp.tile([P, JN], f32,
                                                tag=f"tmp{b2 * JT + j4}",
                                                name=f"tmp{b}_{j4}")
                                nc.vector.tensor_tensor(
                                    tmp[:], ps[(b2, j4)][:],
                                    thb[:, jj:jj + JN], op=ALU.subtract)
                                tmps.append((b, jj, tmp))
                            else:
                                nc.vector.tensor_tensor(
                                    obs[b][:, jj:jj + JN], ps[(b2, j4)][:],
                                    bound[(b, j4)][:], op=ALU.is_gt)
                                nc.sync.dma_start(o_d[b, :, jj:jj + JN],
                                                  obs[b][:, jj:jj + JN])
                    if w == 0:
                        # b2/b3 rowsums, wave-0 op2s, wave-1 bound tiles
                        # -- all on DVE, emitted after wave-0's op1s so
                        # they can't head-of-line block the psum-bank
                        # releases.
                        for i, b in enumerate((2, 3)):
                            nc.vector.tensor_scalar(
                                sc8[2][:], xrm23[:, i], 1.0, 0.0,
                                op0=ALU.mult, op1=ALU.add,
                                accum_out=rxa[b][:])
                            nc.vector.tensor_scalar(
                                rxe[b][:], rxa[b][:], 1.0 / 1024.0, -4.0,
                                op0=ALU.mult, op1=ALU.add)
                        for b, jj, tmp in tmps:
                            nc.vector.tensor_scalar(
                                obs[b][:, jj:jj + JN], tmp[:],
                                rxe[b][:], None, op0=ALU.is_gt)
                            nc.sync.dma_start(o_d[b, :, jj:jj + JN],
                                              obs[b][:, jj:jj + JN])
                        bound = {}
                        for b in (2, 3):
                            for j4 in range(JT):
                                bt = boundp.tile([P, JN], f32,
                                                 name=f"bnd{b}_{j4}")
                                nc.vector.tensor_scalar(
                                    bt[:], thb[:, j4 * JN:(j4 + 1) * JN],
                                    rxe[b][:], None, op0=ALU.add)
                                bound[(b, j4)] = bt

    nc.compile()
    return nc


def _get_nc():
    if "nc" not in _cache:
        _cache["nc"] = _build()
    return _cache["nc"]


def _prep_core(xs8, mask_buf, thb_buf):
    """Per-core input dict from the fp8 x slice and shared mask/th bufs."""
    t = xs8.reshape(NB, P, KP, 2, P)            # [b, m, kp, ko, ki]
    xT_buf = np.ascontiguousarray(t.transpose(0, 4, 2, 3, 1))
    return {
        "xT": xT_buf,                            # [b, ki, kp, ko, m]
        "xrm": np.ascontiguousarray(xs8.reshape(NB, P, D)),
        "masks": mask_buf,
        "thb": thb_buf,
    }


def run(x, masks, thresholds, trace=False):
    """Run the SPMD kernel on 8 cores. Returns (out_bool, results)."""
    import ml_dtypes
    from concourse.bass_utils import run_bass_kernel_spmd

    nc = _get_nc()
    f8 = ml_dtypes.float8_e4m3

    xs8_all = np.where(np.asarray(x) != 0, np.float32(1.0),
                       np.float32(-1.0)).astype(f8)
    m_u8 = np.ascontiguousarray(np.asarray(masks).view(np.uint8))
    th = np.asarray(thresholds).astype(np.float32) * np.float32(2.0 ** -9)

    mask_bufs, thb_bufs = [], []
    for h in range(GJ):
        mh = m_u8[:, h * JL:(h + 1) * JL].reshape(KP, 2, P, JL)
        mask_bufs.append(np.ascontiguousarray(mh.transpose(0, 2, 1, 3)))
        thb_bufs.append(np.ascontiguousarray(
            np.broadcast_to(th[None, h * JL:(h + 1) * JL], (P, JL))))

    in_maps = []
    for c in range(NCORES):
        g, h = c // GJ, c % GJ
        in_maps.append(_prep_core(xs8_all[g * ML:(g + 1) * ML],
                                  mask_bufs[h], thb_bufs[h]))

    res = run_bass_kernel_spmd(nc, in_maps, core_ids=list(range(NCORES)),
                               trace=trace)
    out = np.empty((B, J), dtype=np.uint8)
    for c in range(NCORES):
        g, h = c // GJ, c % GJ
        out[g * ML:(g + 1) * ML, h * JL:(h + 1) * JL] = \
            res.results[c]["out"].reshape(ML, JL)
    return out.view(np.bool_), res


def kernel(x, masks, thresholds):
    x = np.asarray(x)
    masks = np.asarray(masks)
    thresholds = np.asarray(thresholds)
    out, _ = run(x, masks, thresholds, trace=False)
    return out
